# revision 46
# baseline (speedup 1.0000x reference)
"""Trainium2 Bass kernel for one backward-Euler implicit 1D diffusion step
(Thomas tridiagonal solve) on an 8,388,608-point grid, distributed over 8
NeuronCores.

Math: the tridiagonal system (I - dt*D*Lap) x = d has constant coefficients
a = c = -r, b = 1+2r with r = D*dt/dx^2 = 0.1 (Dirichlet rows at the two
ends).  The matrix is strongly diagonally dominant, so rows of its inverse
decay geometrically (ratio lam ~ 0.084 per step).  To the required accuracy
the solve is therefore a 9-tap symmetric FIR convolution of the RHS
(truncation tail ~1e-5 relative), except within ~30 points of the two
global boundaries, which are recomputed exactly on the host (the trivially
small "reduced interface system" of the domain-decomposition approach).

Device implementation (overlap-save, fp16): each core owns a contiguous
1,048,576-point chunk.  The host shards it into overlapping 128-point
windows with stride S = 128 - 2K = 120 and casts to fp16: the input stream
carries the banded 128x128 fp16 weight matrix W (W[p, i] = w[p-K-i]) in its
first 128 columns followed by R[p, f] = d[S*f + p - K], so the weights ride
the same 4 KB-packet DMA chunks as the data (a separate 128x256 B transfer
would clog the queue's descriptor dispatch for ~3.5 us).  One full-rate
TensorE matmul pass (1 cycle/row fp16) computes all S=120 valid outputs of
every window: out[i, f] = x[S*f + i].  Input and output are split into
position-ordered ~1024-column chunks alternating between the two HWDGE
rings (sync + scalar, the input tail on SWDGE) so the column frontier
advances uniformly and compute + output stores chase the input stream; the
~4.4 MB per core of DMA traffic streams at the ~390 GB/s shared 16-engine
DMA pool rate (the per-core roofline), and the remaining span is the fixed
~8.5 us NEFF preamble and ~8.6 us teardown barrier of the runtime.
"""

from contextlib import ExitStack

import numpy as np

import concourse.bacc as bacc
import concourse.mybir as mybir
import concourse.tile as tile

N = 8_388_608
NCORES = 8
P = 128
PER_CORE = N // NCORES            # 1,048,576
K = 4                             # FIR radius (9 taps); also keeps S = 120 a
                                  # multiple of 8 - a 124-row store falls off
                                  # the fast DMA path onto a 4-engine pool
S = P - 2 * K                     # 120 valid outputs per window
NCOLS = -(-PER_CORE // S)         # 8,739 windows per core
NF = 512                          # max matmul moving free dim (one PSUM bank)
FIX = 512                         # host boundary fix-up length
ECOLS = P + NCOLS                 # weights (128 cols) + window columns

# matmul group sizes along the window axis
GROUP_SIZES = [NF] * (NCOLS // NF) + ([NCOLS % NF] if NCOLS % NF else [])

# position-ordered DMA chunking, group-aligned, alternating sync/scalar.
# Chunk widths are small at both ends of the stream (fast per-chunk
# completion exactly when the compute pipeline is latency-bound) and large
# in the middle (fewer triggers at ~0.85 us engine time each and fewer
# per-engine completion markers); ring entries are 128+16 per chunk
# regardless of width, so wide middle chunks also ease ring credit.
IN_EDGES = [0, P + 2 * NF] + [P + k * NF for k in range(4, 18, 2)] + [ECOLS]
OUT_EDGES = [k * NF for k in range(0, 17, 2)] + [NCOLS]

# stash of the last BassKernelResults for test harnesses
LAST_RESULTS = None


def _coeffs(dt):
    """fp32 tridiagonal coefficients exactly as the reference computes them."""
    dtf = np.float32(dt)
    r = np.float32(np.float32(1e-9) * dtf) / np.float32(1e-4 * 1e-4)
    a = np.float32(-r)
    b = np.float32(np.float32(1.0) + np.float32(2.0) * r)
    c = np.float32(-r)
    return r, a, b, c


def _fir_taps(a, b, c):
    """Centered row of inv(tridiag(a,b,c)) in fp64: the 2K+1 FIR taps."""
    M = 4096
    af, bf, cf = float(a), float(b), float(c)
    d = np.zeros(M)
    d[M // 2] = 1.0
    cp = np.empty(M)
    dp = np.empty(M)
    cp[0] = cf / bf
    dp[0] = d[0] / bf
    for i in range(1, M):
        den = bf - af * cp[i - 1]
        cp[i] = cf / den
        dp[i] = (d[i] - af * dp[i - 1]) / den
    x = np.empty(M)
    x[-1] = dp[-1]
    for i in range(M - 2, -1, -1):
        x[i] = dp[i] - cp[i] * x[i + 1]
    return x[M // 2 - K : M // 2 + K + 1]


def _weight_mat(w):
    """Banded lhsT weight matrix: out[i,f] = sum_p W[p,i] R[p,f]."""
    W = np.zeros((P, P), dtype=np.float64)
    for p in range(P):
        for i in range(S):
            j = p - K - i
            if -K <= j <= K:
                W[p, i] = w[j + K]
    return W.astype(np.float16)


def _build_device_program():
    nc = bacc.Bacc("TRN2", debug=False)
    R = nc.dram_tensor("r_in", [P, ECOLS], mybir.dt.float16, kind="ExternalInput")
    X = nc.dram_tensor("x_out", [S, NCOLS], mybir.dt.float16, kind="ExternalOutput")

    with tile.TileContext(nc) as tc, ExitStack() as ctx:
        epool = ctx.enter_context(tc.tile_pool(name="e", bufs=1))
        psum = ctx.enter_context(tc.tile_pool(name="ps", bufs=6, space="PSUM"))
        opool = ctx.enter_context(tc.tile_pool(name="o", bufs=1))

        # input (weights in cols 0:128, then window data): position-ordered
        # chunks alternating between the two HWDGE rings
        e_t = epool.tile([P, ECOLS], mybir.dt.float16)
        # input chunks alternate the two HWDGE rings in position order so the
        # column frontier advances uniformly; the small final chunk rides
        # SWDGE so the input tail is never stuck behind a ring-credit stall.
        in_engines = [nc.sync, nc.scalar, nc.sync, nc.scalar, nc.sync,
                      nc.scalar, nc.sync, nc.scalar, nc.gpsimd]
        for eng, (lo, hi) in zip(in_engines, zip(IN_EDGES, IN_EDGES[1:])):
            eng.dma_start(e_t[:, lo:hi], R[:, lo:hi])

        w_t = e_t[:, 0:P]

        # one big output SBUF tile (valid rows 0..S), flushed in chunks as
        # soon as the covering copies land, alternating rings; each trigger
        # costs ~0.85 us of engine execution (120 descriptors)
        # output chunks spread over BOTH HW rings plus SWDGE for three middle
        # chunks: stores dispatch slower than loads (~250 vs ~390 GB/s), so a
        # third descriptor stream through the SW queue widens the write-only
        # end of the stream; SWDGE's ~3 us descgen latency is hidden because
        # those chunks are ready mid-stream
        o_t = opool.tile([P, NCOLS], mybir.dt.float16)
        out_engines = [nc.scalar, nc.gpsimd, nc.sync, nc.gpsimd, nc.scalar,
                       nc.gpsimd, nc.sync, nc.scalar, nc.sync]

        oi = 0
        c0 = 0
        for g, gw in enumerate(GROUP_SIZES):
            ps = psum.tile([P, NF], mybir.dt.float32, tag="ps")
            nc.tensor.matmul(
                ps[:, :gw], w_t, e_t[:, P + c0 : P + c0 + gw], start=True, stop=True
            )
            dst = o_t[:S, c0 : c0 + gw]
            # PSUM->SBUF (fp32 -> fp16) copies, 2/3 Vector 1/3 Scalar so the
            # scalar engine keeps room for its extra DMA triggers
            if g % 3 == 2:
                nc.scalar.activation(dst, ps[:S, :gw], mybir.ActivationFunctionType.Copy)
            else:
                nc.vector.tensor_copy(dst, ps[:S, :gw])
            c0 += gw
            if c0 >= OUT_EDGES[oi + 1]:
                lo, hi = OUT_EDGES[oi], OUT_EDGES[oi + 1]
                out_engines[oi].dma_start(X[:, lo:hi], o_t[:S, lo:hi])
                oi += 1
    nc.compile()
    return nc


def _host_fixup(x, C, a, b, c, C_surf, C_bulk):
    """Exact fp32 reference recurrences for the first/last FIX points."""
    n = x.shape[0]
    # left end: exact forward elimination from the Dirichlet row 0
    d0 = C[: FIX + 1].astype(np.float32).copy()
    d0[0] = C_surf
    cp = np.empty(FIX + 1, dtype=np.float32)
    dp = np.empty(FIX + 1, dtype=np.float32)
    cp[0] = np.float32(0.0)
    dp[0] = np.float32(C_surf)
    for i in range(1, FIX + 1):
        den = np.float32(b - a * cp[i - 1])
        cp[i] = np.float32(c / den)
        dp[i] = np.float32((d0[i] - a * dp[i - 1]) / den)
    xl = np.empty(FIX + 1, dtype=np.float32)
    xl[FIX] = x[FIX]
    for i in range(FIX - 1, -1, -1):
        xl[i] = np.float32(dp[i] - cp[i] * xl[i + 1])
    x[:FIX] = xl[:FIX]

    # right end: converged forward state (warmed up), Dirichlet last row
    cpc = np.float32(0.0)
    for _ in range(200):
        den = np.float32(b - a * cpc)
        cpc = np.float32(c / den)
    den_star = np.float32(b - a * cpc)
    warm = 64
    start = n - FIX - warm
    dp_t = np.empty(FIX + 1, dtype=np.float32)
    st = np.float32(0.0)
    for i in range(start, n - 1):
        st = np.float32((np.float32(C[i]) - a * st) / den_star)
        if i >= n - 1 - FIX:
            dp_t[i - (n - 1 - FIX)] = st
    dp_t[FIX] = np.float32(C_bulk)
    xr = np.empty(FIX + 1, dtype=np.float32)
    xr[FIX] = dp_t[FIX]
    for k in range(FIX - 1, -1, -1):
        xr[k] = np.float32(dp_t[k] - cpc * xr[k + 1])
    x[n - 1 - FIX :] = xr
    return x


def kernel(C, dt, C_surf, C_bulk):
    from concourse.bass_utils import run_bass_kernel_spmd

    global LAST_RESULTS

    C = np.asarray(C, dtype=np.float32).reshape(-1)
    assert C.shape[0] == N
    cs = np.float32(np.asarray(C_surf))
    cb = np.float32(np.asarray(C_bulk))
    r, a, b, c = _coeffs(np.asarray(dt))

    w = _fir_taps(a, b, c)
    W = _weight_mat(w)

    # ---- shard: pad + Dirichlet rows, cast fp16, then per-core overlapping
    # windows prefixed by the weight block:
    #   r_in[:, 0:128]   = W
    #   r_in[p, 128 + f] = d[core*PER_CORE + S*f + p - K]
    d_pad = np.zeros(N + 2 * P, dtype=np.float32)
    d_pad[P : P + N] = C
    d_pad[P] = cs               # Dirichlet row 0:    d[0]   -> C_surf
    d_pad[P + N - 1] = cb       # Dirichlet row N-1:  d[N-1] -> C_bulk
    d_pad16 = d_pad.astype(np.float16)

    in_maps = []
    for cidx in range(NCORES):
        base = P + cidx * PER_CORE - K
        Rv = np.lib.stride_tricks.as_strided(
            d_pad16[base:], shape=(NCOLS, P), strides=(S * 2, 2)
        )
        r_in = np.empty((P, ECOLS), dtype=np.float16)
        r_in[:, :P] = W
        r_in[:, P:] = Rv.T
        in_maps.append({"r_in": r_in})

    nc = _build_device_program()
    res = run_bass_kernel_spmd(nc, in_maps, core_ids=list(range(NCORES)))
    LAST_RESULTS = res

    # ---- gather: x[S*f + i] = out[i, f]
    x = np.empty(N, dtype=np.float32)
    for cidx in range(NCORES):
        out = res.results[cidx]["x_out"]  # (120, 8739) fp16
        x[cidx * PER_CORE : (cidx + 1) * PER_CORE] = (
            np.ascontiguousarray(out.T).astype(np.float32).reshape(-1)[:PER_CORE]
        )

    return _host_fixup(x, C, a, b, c, cs, cb)


# revision 47
# speedup vs baseline: 1.1077x; 1.1077x over previous
"""Trainium2 Bass kernel for one backward-Euler implicit 1D diffusion step
(Thomas tridiagonal solve) on an 8,388,608-point grid, distributed over 8
NeuronCores.

Math: the tridiagonal system (I - dt*D*Lap) x = d has constant coefficients
a = c = -r, b = 1+2r with r = D*dt/dx^2 = 0.1 (Dirichlet rows at the two
ends).  The matrix is strongly diagonally dominant, so rows of its inverse
decay geometrically (ratio lam ~ 0.084 per step).  To the required accuracy
the solve is therefore a 9-tap symmetric FIR convolution of the RHS
(truncation tail ~1e-5 relative), except within ~30 points of the two
global boundaries, which are recomputed exactly on the host (the trivially
small "reduced interface system" of the domain-decomposition approach).

Device implementation (overlap-save, fp16): each core owns a contiguous
1,048,576-point chunk.  The host shards it into overlapping 128-point
windows with stride S = 128 - 2K = 120 and casts to fp16: the input stream
carries the banded 128x128 fp16 weight matrix W (W[p, i] = w[p-K-i]) in its
first 128 columns followed by R[p, f] = d[S*f + p - K], so the weights ride
the same 4 KB-packet DMA chunks as the data (a separate 128x256 B transfer
would clog the queue's descriptor dispatch for ~3.5 us).  One full-rate
TensorE matmul pass (1 cycle/row fp16) computes all S=120 valid outputs of
every window: out[i, f] = x[S*f + i].  Input and output are split into
position-ordered ~1024-column chunks alternating between the two HWDGE
rings (sync + scalar, the input tail on SWDGE) so the column frontier
advances uniformly and compute + output stores chase the input stream; the
~4.4 MB per core of DMA traffic streams at the ~390 GB/s shared 16-engine
DMA pool rate (the per-core roofline), and the remaining span is the fixed
~8.5 us NEFF preamble and ~8.6 us teardown barrier of the runtime.
"""

from contextlib import ExitStack

import numpy as np

import concourse.bacc as bacc
import concourse.mybir as mybir
import concourse.tile as tile

N = 8_388_608
NCORES = 8
P = 128
PER_CORE = N // NCORES            # 1,048,576
K = 4                             # FIR radius (9 taps); also keeps S = 120 a
                                  # multiple of 8 - a 124-row store falls off
                                  # the fast DMA path onto a 4-engine pool
S = P - 2 * K                     # 120 valid outputs per window
NCOLS = -(-PER_CORE // S)         # 8,739 windows per core
NF = 512                          # max matmul moving free dim (one PSUM bank)
FIX = 512                         # host boundary fix-up length
ECOLS = P + NCOLS                 # weights (128 cols) + window columns

# matmul group sizes along the window axis
GROUP_SIZES = [NF] * (NCOLS // NF) + ([NCOLS % NF] if NCOLS % NF else [])

# position-ordered DMA chunking, group-aligned, alternating sync/scalar.
# Chunk widths are small at both ends of the stream (fast per-chunk
# completion exactly when the compute pipeline is latency-bound) and large
# in the middle (fewer triggers at ~0.85 us engine time each and fewer
# per-engine completion markers); ring entries are 128+16 per chunk
# regardless of width, so wide middle chunks also ease ring credit.
IN_EDGES = [0, P + 2 * NF] + [P + k * NF for k in range(4, 18, 2)] + [ECOLS]
OUT_EDGES = [k * NF for k in range(0, 17, 2)] + [NCOLS]

# stash of the last BassKernelResults for test harnesses
LAST_RESULTS = None


def _coeffs(dt):
    """fp32 tridiagonal coefficients exactly as the reference computes them."""
    dtf = np.float32(dt)
    r = np.float32(np.float32(1e-9) * dtf) / np.float32(1e-4 * 1e-4)
    a = np.float32(-r)
    b = np.float32(np.float32(1.0) + np.float32(2.0) * r)
    c = np.float32(-r)
    return r, a, b, c


def _fir_taps(a, b, c):
    """Centered row of inv(tridiag(a,b,c)) in fp64: the 2K+1 FIR taps."""
    M = 4096
    af, bf, cf = float(a), float(b), float(c)
    d = np.zeros(M)
    d[M // 2] = 1.0
    cp = np.empty(M)
    dp = np.empty(M)
    cp[0] = cf / bf
    dp[0] = d[0] / bf
    for i in range(1, M):
        den = bf - af * cp[i - 1]
        cp[i] = cf / den
        dp[i] = (d[i] - af * dp[i - 1]) / den
    x = np.empty(M)
    x[-1] = dp[-1]
    for i in range(M - 2, -1, -1):
        x[i] = dp[i] - cp[i] * x[i + 1]
    return x[M // 2 - K : M // 2 + K + 1]


def _weight_mat(w):
    """Banded lhsT weight matrix: out[i,f] = sum_p W[p,i] R[p,f]."""
    W = np.zeros((P, P), dtype=np.float64)
    for p in range(P):
        for i in range(S):
            j = p - K - i
            if -K <= j <= K:
                W[p, i] = w[j + K]
    return W.astype(np.float16)


def _build_device_program():
    nc = bacc.Bacc("TRN2", debug=False)
    R = nc.dram_tensor("r_in", [P, ECOLS], mybir.dt.float16, kind="ExternalInput")
    X = nc.dram_tensor("x_out", [S, NCOLS], mybir.dt.float16, kind="ExternalOutput")

    with tile.TileContext(nc) as tc, ExitStack() as ctx:
        epool = ctx.enter_context(tc.tile_pool(name="e", bufs=1))
        psum = ctx.enter_context(tc.tile_pool(name="ps", bufs=6, space="PSUM"))
        opool = ctx.enter_context(tc.tile_pool(name="o", bufs=1))

        # input (weights in cols 0:128, then window data): position-ordered
        # chunks alternating between the two HWDGE rings
        e_t = epool.tile([P, ECOLS], mybir.dt.float16)
        # input chunks alternate the two HWDGE rings in position order so the
        # column frontier advances uniformly; the small final chunk rides
        # SWDGE so the input tail is never stuck behind a ring-credit stall.
        in_engines = [nc.sync, nc.scalar, nc.sync, nc.scalar, nc.sync,
                      nc.scalar, nc.sync, nc.scalar, nc.gpsimd]
        for eng, (lo, hi) in zip(in_engines, zip(IN_EDGES, IN_EDGES[1:])):
            eng.dma_start(e_t[:, lo:hi], R[:, lo:hi])

        w_t = e_t[:, 0:P]

        # one big output SBUF tile (valid rows 0..S), flushed in chunks as
        # soon as the covering copies land, alternating rings; each trigger
        # costs ~0.85 us of engine execution (120 descriptors)
        # output chunks spread over BOTH HW rings plus SWDGE for three middle
        # chunks: stores dispatch slower than loads (~250 vs ~390 GB/s), so a
        # third descriptor stream through the SW queue widens the write-only
        # end of the stream; SWDGE's ~3 us descgen latency is hidden because
        # those chunks are ready mid-stream
        o_t = opool.tile([P, NCOLS], mybir.dt.float16)
        out_engines = [nc.scalar, nc.sync, nc.gpsimd, nc.scalar, nc.gpsimd,
                       nc.sync, nc.gpsimd, nc.scalar, nc.sync]

        oi = 0
        c0 = 0
        for g, gw in enumerate(GROUP_SIZES):
            ps = psum.tile([P, NF], mybir.dt.float32, tag="ps")
            nc.tensor.matmul(
                ps[:, :gw], w_t, e_t[:, P + c0 : P + c0 + gw], start=True, stop=True
            )
            dst = o_t[:S, c0 : c0 + gw]
            # PSUM->SBUF (fp32 -> fp16) copies, 2/3 Vector 1/3 Scalar so the
            # scalar engine keeps room for its extra DMA triggers
            if g % 3 == 2:
                nc.scalar.activation(dst, ps[:S, :gw], mybir.ActivationFunctionType.Copy)
            else:
                nc.vector.tensor_copy(dst, ps[:S, :gw])
            c0 += gw
            if c0 >= OUT_EDGES[oi + 1]:
                lo, hi = OUT_EDGES[oi], OUT_EDGES[oi + 1]
                out_engines[oi].dma_start(X[:, lo:hi], o_t[:S, lo:hi])
                oi += 1
    nc.compile()
    return nc


def _host_fixup(x, C, a, b, c, C_surf, C_bulk):
    """Exact fp32 reference recurrences for the first/last FIX points."""
    n = x.shape[0]
    # left end: exact forward elimination from the Dirichlet row 0
    d0 = C[: FIX + 1].astype(np.float32).copy()
    d0[0] = C_surf
    cp = np.empty(FIX + 1, dtype=np.float32)
    dp = np.empty(FIX + 1, dtype=np.float32)
    cp[0] = np.float32(0.0)
    dp[0] = np.float32(C_surf)
    for i in range(1, FIX + 1):
        den = np.float32(b - a * cp[i - 1])
        cp[i] = np.float32(c / den)
        dp[i] = np.float32((d0[i] - a * dp[i - 1]) / den)
    xl = np.empty(FIX + 1, dtype=np.float32)
    xl[FIX] = x[FIX]
    for i in range(FIX - 1, -1, -1):
        xl[i] = np.float32(dp[i] - cp[i] * xl[i + 1])
    x[:FIX] = xl[:FIX]

    # right end: converged forward state (warmed up), Dirichlet last row
    cpc = np.float32(0.0)
    for _ in range(200):
        den = np.float32(b - a * cpc)
        cpc = np.float32(c / den)
    den_star = np.float32(b - a * cpc)
    warm = 64
    start = n - FIX - warm
    dp_t = np.empty(FIX + 1, dtype=np.float32)
    st = np.float32(0.0)
    for i in range(start, n - 1):
        st = np.float32((np.float32(C[i]) - a * st) / den_star)
        if i >= n - 1 - FIX:
            dp_t[i - (n - 1 - FIX)] = st
    dp_t[FIX] = np.float32(C_bulk)
    xr = np.empty(FIX + 1, dtype=np.float32)
    xr[FIX] = dp_t[FIX]
    for k in range(FIX - 1, -1, -1):
        xr[k] = np.float32(dp_t[k] - cpc * xr[k + 1])
    x[n - 1 - FIX :] = xr
    return x


def kernel(C, dt, C_surf, C_bulk):
    from concourse.bass_utils import run_bass_kernel_spmd

    global LAST_RESULTS

    C = np.asarray(C, dtype=np.float32).reshape(-1)
    assert C.shape[0] == N
    cs = np.float32(np.asarray(C_surf))
    cb = np.float32(np.asarray(C_bulk))
    r, a, b, c = _coeffs(np.asarray(dt))

    w = _fir_taps(a, b, c)
    W = _weight_mat(w)

    # ---- shard: pad + Dirichlet rows, cast fp16, then per-core overlapping
    # windows prefixed by the weight block:
    #   r_in[:, 0:128]   = W
    #   r_in[p, 128 + f] = d[core*PER_CORE + S*f + p - K]
    d_pad = np.zeros(N + 2 * P, dtype=np.float32)
    d_pad[P : P + N] = C
    d_pad[P] = cs               # Dirichlet row 0:    d[0]   -> C_surf
    d_pad[P + N - 1] = cb       # Dirichlet row N-1:  d[N-1] -> C_bulk
    d_pad16 = d_pad.astype(np.float16)

    in_maps = []
    for cidx in range(NCORES):
        base = P + cidx * PER_CORE - K
        Rv = np.lib.stride_tricks.as_strided(
            d_pad16[base:], shape=(NCOLS, P), strides=(S * 2, 2)
        )
        r_in = np.empty((P, ECOLS), dtype=np.float16)
        r_in[:, :P] = W
        r_in[:, P:] = Rv.T
        in_maps.append({"r_in": r_in})

    nc = _build_device_program()
    res = run_bass_kernel_spmd(nc, in_maps, core_ids=list(range(NCORES)))
    LAST_RESULTS = res

    # ---- gather: x[S*f + i] = out[i, f]
    x = np.empty(N, dtype=np.float32)
    for cidx in range(NCORES):
        out = res.results[cidx]["x_out"]  # (120, 8739) fp16
        x[cidx * PER_CORE : (cidx + 1) * PER_CORE] = (
            np.ascontiguousarray(out.T).astype(np.float32).reshape(-1)[:PER_CORE]
        )

    return _host_fixup(x, C, a, b, c, cs, cb)
